# revision 1
# baseline (speedup 1.0000x reference)
"""Trainium2 Bass kernel for nn_ConsistentSelfAttentionProcessor.

Reference computation (per frame-set of NUM_FRAMES=4 frames):
    q,k,v = hs@Wq+bq, hs@Wk+bk, hs@Wv+bv          # [BF,S,D]
    per head: K_comb = [K(frame0_of_set); K(own)]  # 2S keys
    out = softmax(q@K_comb^T/sqrt(hd)) @ V_comb @ Wo + bo + hs

Sharding: 8 cores = 2 frame-sets x 4 head-groups of 5 heads.
Each core computes a partial output  attn(set, heads_g) @ Wo[rows_g]  in bf16;
the host sums the 4 per-set partials in fp32 and adds bo + residual.

Frame 0 of each set attends to [K0;K0] which equals softmax over K0 alone,
so frame 0 uses 1024 keys instead of 2048.

Softmax uses no max subtraction: scores*0.125 is bounded (~|3|) for these
inputs, so exp is safe in fp32. The softmax denominator comes for free from a
ones-column appended to V (U_T row 64 = sum(exp)).
"""

import sys
from contextlib import ExitStack

import numpy as np

sys.path.insert(0, "/opt/trn_rl_repo")

import ml_dtypes  # noqa: E402

import concourse.bass as bass  # noqa: E402
import concourse.mybir as mybir  # noqa: E402
import concourse.tile as tile  # noqa: E402
from concourse import bacc, bass_utils  # noqa: E402
from concourse.masks import make_identity  # noqa: E402

BF16 = mybir.dt.bfloat16
F32 = mybir.dt.float32
NPBF16 = ml_dtypes.bfloat16

NUM_FRAMES = 4
HEADS = 20
BF, S, D = 8, 1024, 1280
HD = 64  # head dim
B = BF // NUM_FRAMES  # 2 frame sets
N_CORES = 8
GROUPS = 4  # head groups per set
HG = HEADS // GROUPS  # 5 heads per group
C = HG * HD  # 320 columns per group
N_SET = NUM_FRAMES * S  # 4096 rows per set
SCALE = 1.0 / np.sqrt(HD)  # 0.125

P = 128
KC_D = D // P  # 10 contraction chunks for projections
TC_N = N_SET // P  # 32 token chunks per set
QH = 2  # q halves of 512 per frame


def build_kernel_body(ctx: ExitStack, tc: tile.TileContext, xt, wqkv, wo, bqkv, out):
    """Emit the per-core program.

    xt:   [D, N_SET]      bf16  (X^T for this set)
    wqkv: [D, 3*C]        bf16  (columns: Wq_g | Wk_g | Wv_g)
    wo:   [3*P, D]        bf16  (rows 0..C-1 = Wo[group rows]; rest zero pad)
    bqkv: [3*C]           f32
    out:  [N_SET, D]      bf16  (partial output, unsummed, no bo/residual)
    """
    nc = tc.nc

    const = ctx.enter_context(tc.tile_pool(name="const", bufs=1))
    persist = ctx.enter_context(tc.tile_pool(name="persist", bufs=1))
    work = ctx.enter_context(tc.tile_pool(name="work", bufs=3))
    psum = ctx.enter_context(tc.tile_pool(name="psum", bufs=1, space="PSUM"))

    # ---- constants ----------------------------------------------------------
    ident = const.tile([P, P], BF16, tag="ident")
    make_identity(nc, ident)
    ones = const.tile([P, P], F32, tag="ones")
    nc.gpsimd.memset(ones, 1.0)

    wqkv_sb = const.tile([P, KC_D, 3 * C], BF16, tag="wqkv")
    nc.sync.dma_start(wqkv_sb, wqkv.rearrange("(c p) n -> p c n", p=P))
    wo_sb = const.tile([P, 3, D], BF16, tag="wo")
    nc.sync.dma_start(wo_sb, wo.rearrange("(c p) n -> p c n", p=P))
    bqkv_sb = const.tile([1, 3 * C], F32, tag="bqkv")
    nc.sync.dma_start(bqkv_sb, bqkv[None, :])

    # broadcast biases across partitions once: bias_bc[p, j] = bqkv[j]
    bias_bc = const.tile([P, 3 * C], F32, tag="bias_bc")
    bps = psum.tile([P, 3 * C], F32, tag="A", bufs=2)
    nc.tensor.matmul(bps[:, 0:512], ones[0:1, :], bqkv_sb[:, 0:512])
    nc.tensor.matmul(bps[:, 512:960], ones[0:1, :], bqkv_sb[:, 512:960])
    nc.vector.tensor_copy(bias_bc, bps)

    # ---- persistent intermediates ------------------------------------------
    # Q^T/K^T, head-transposed: chunk h//2 holds head pair, base (h%2)*64.
    # chunks 0-2: q-heads, 3-5: k-heads (halves of chunks 2 and 5 unused).
    qkt = persist.tile([P, 6, N_SET], BF16, tag="qkt")
    # V rows with a ones column per head: [tokens, head, 65]
    vsb = persist.tile([P, TC_N, HG, HD + 1], BF16, tag="vsb")
    nc.gpsimd.memset(vsb[:, :, :, HD], 1.0)
    # attn^T for O-proj, one tensor per frame so O-proj(f) only depends on
    # frame f's attention: chunk c holds heads (2c, 2c+1); chunk 2 half unused
    atn_f = [
        persist.tile([P, 3, S], BF16, tag=f"atn{f}", name=f"atn{f}")
        for f in range(NUM_FRAMES)
    ]
    for f in range(NUM_FRAMES):
        nc.gpsimd.memset(atn_f[f][64:128, 2, :], 0.0)

    # ---- phase 1: QKV projections ------------------------------------------
    for t in range(TC_N):
        xcol = work.tile([P, KC_D, P], BF16, tag="xcol")
        nc.sync.dma_start(
            xcol, xt[:, t * P : (t + 1) * P].rearrange("(c p) n -> p c n", p=P)
        )
        pq = psum.tile([P, 3 * C], F32, tag="A", bufs=2)
        for kc in range(KC_D):
            st, sp = kc == 0, kc == KC_D - 1
            nc.tensor.matmul(
                pq[:, 0:512], xcol[:, kc], wqkv_sb[:, kc, 0:512], start=st, stop=sp
            )
            nc.tensor.matmul(
                pq[:, 512:960], xcol[:, kc], wqkv_sb[:, kc, 512:960], start=st, stop=sp
            )
        # V part: bias add + split per head into vsb
        nc.vector.tensor_tensor(
            vsb[:, t, :, 0:HD],
            pq[:, 2 * C : 3 * C].rearrange("p (h d) -> p h d", d=HD),
            bias_bc[:, 2 * C : 3 * C].rearrange("p (h d) -> p h d", d=HD),
            mybir.AluOpType.add,
        )
        # QK part: bias add + cast, then PE-transpose into qkt
        rows = work.tile([P, 2 * C], BF16, tag="rows")
        nc.vector.tensor_tensor(
            rows, pq[:, 0 : 2 * C], bias_bc[:, 0 : 2 * C], mybir.AluOpType.add
        )
        # 6 transposes: (q0q1)(q2q3)(q4)(k0k1)(k2k3)(k4)
        for ch in range(6):
            width = HD if ch in (2, 5) else P
            src = rows[:, ch * P : ch * P + width] if ch < 3 else rows[
                :, C + (ch - 3) * P : C + (ch - 3) * P + width
            ]
            tp = psum.tile([P, P], BF16, tag="C", bufs=2)
            nc.tensor.transpose(tp[0:width, :], src, ident)
            nc.vector.tensor_copy(qkt[0:width, ch, t * P : (t + 1) * P], tp[0:width, :])

    # ---- phase 2+3: attention, O-proj per frame -----------------------------
    for f in range(NUM_FRAMES):
        qoff = f * S
        nkc = 8 if f == 0 else 16  # frame 0: ref==own, dedup
        for h in range(HG):
            b = (h % 2) * HD  # partition base for this head
            qch = h // 2
            kch = 3 + h // 2
            ut = psum.tile([P, S], F32, tag="ut", bufs=1)
            for kc in range(nkc):
                # key token position: first 8 chunks ref frame, rest own frame
                ktok = kc * P if kc < 8 else qoff + (kc - 8) * P
                sc = psum.tile([P, S], F32, tag="A", bufs=2)
                for q in range(QH):
                    nc.tensor.matmul(
                        sc[:, q * 512 : (q + 1) * 512],
                        qkt[b : b + HD, kch, ktok : ktok + P],
                        qkt[b : b + HD, qch, qoff + q * 512 : qoff + (q + 1) * 512],
                    )
                ex = work.tile([P, S], BF16, tag="ex")
                nc.scalar.activation(
                    ex, sc, mybir.ActivationFunctionType.Exp, scale=SCALE
                )
                for q in range(QH):
                    nc.tensor.matmul(
                        ut[0 : HD + 1, q * 512 : (q + 1) * 512],
                        vsb[:, ktok // P, h, :],
                        ex[:, q * 512 : (q + 1) * 512],
                        start=(kc == 0),
                        stop=(kc == nkc - 1),
                    )
            # normalize: attn^T = ut[0:64] / bcast(ut[64]).  Keep PE out of
            # this tail: DVE copies s to SBUF, idle GpSimd broadcasts it
            # across partitions, DVE divides (single PSUM operand rule ok).
            rc = work.tile([HD + 1, S], F32, tag="rc", bufs=2)
            nc.vector.reciprocal(rc[HD : HD + 1, :], ut[HD : HD + 1, :])
            for q in range(QH):
                qs = slice(q * 512, (q + 1) * 512)
                bcp = psum.tile([HD, 512], F32, tag="C", bufs=2)
                nc.tensor.matmul(bcp, ones[HD : HD + 1, 0:HD], rc[HD : HD + 1, qs])
                # DVE can read only one PSUM operand per op: stage via SBUF
                bc = work.tile([HD, 512], F32, tag="bcs", bufs=2)
                nc.vector.tensor_copy(bc, bcp)
                if h % 2 == 0:
                    nc.vector.tensor_tensor(
                        atn_f[f][0:HD, h // 2, q * 512 : (q + 1) * 512],
                        ut[0:HD, qs],
                        bc,
                        mybir.AluOpType.mult,
                    )
                else:
                    # result must land at partitions 64-127: mult to a base-0
                    # tmp, then PE-copy shifts partitions
                    tm = work.tile([HD, 512], BF16, tag="tm", bufs=2)
                    nc.vector.tensor_tensor(tm, ut[0:HD, qs], bc, mybir.AluOpType.mult)
                    pc = psum.tile([P, 512], F32, tag="C", bufs=2)
                    nc.tensor.matmul(pc[HD:P, :], ident[0:HD, 0:HD], tm)
                    nc.vector.tensor_copy(
                        atn_f[f][HD:P, h // 2, q * 512 : (q + 1) * 512],
                        pc[HD:P, :],
                    )
        # O-proj for this frame's 8 token chunks (fills ACT-bound gaps of the
        # next frame's attention on PE)
        for tl in range(S // P):
            t = f * (S // P) + tl
            ou = work.tile([P, D], BF16, tag="ou")
            for n3, nw in ((0, 512), (1, 512), (2, 256)):
                po = psum.tile([P, 512], F32, tag="A", bufs=2)
                for kc in range(3):
                    nc.tensor.matmul(
                        po[:, 0:nw],
                        atn_f[f][:, kc, tl * P : (tl + 1) * P],
                        wo_sb[:, kc, n3 * 512 : n3 * 512 + nw],
                        start=(kc == 0),
                        stop=(kc == 2),
                    )
                nc.vector.tensor_copy(ou[:, n3 * 512 : n3 * 512 + nw], po[:, 0:nw])
            nc.sync.dma_start(out[t * P : (t + 1) * P, :], ou)


def build_program():
    from concourse.bass_interp import get_hw_module

    nc = bacc.Bacc(
        "TRN2",
        target_bir_lowering=False,
        debug=False,
        enable_asserts=False,
        num_devices=N_CORES,
    )
    xt = nc.dram_tensor("xt", [D, N_SET], BF16, kind="ExternalInput").ap()
    wqkv = nc.dram_tensor("wqkv", [D, 3 * C], BF16, kind="ExternalInput").ap()
    wo = nc.dram_tensor("wo", [3 * P, D], BF16, kind="ExternalInput").ap()
    bqkv = nc.dram_tensor("bqkv", [3 * C], F32, kind="ExternalInput").ap()
    out = nc.dram_tensor("out", [N_SET, D], BF16, kind="ExternalOutput").ap()
    with tile.TileContext(nc) as tc:
        with ExitStack() as ctx:
            build_kernel_body(ctx, tc, xt, wqkv, wo, bqkv, out)
    nc.finalize()
    nc.m = get_hw_module(nc.m)
    return nc


def make_in_maps(hidden_states, Wq, Wk, Wv, bq, bk, bv):
    """Per-core inputs. Core c = set (c//4), head group (c%4)."""
    hs = np.asarray(hidden_states, np.float32).reshape(BF, S, D)
    in_maps = []
    xts = []
    for s in range(B):
        x = hs[s * NUM_FRAMES : (s + 1) * NUM_FRAMES].reshape(N_SET, D)
        xts.append(np.ascontiguousarray(x.T).astype(NPBF16))
    for c in range(N_CORES):
        s, g = c // GROUPS, c % GROUPS
        cols = slice(g * C, (g + 1) * C)
        wqkv = np.concatenate(
            [np.asarray(W, np.float32)[:, cols] for W in (Wq, Wk, Wv)], axis=1
        ).astype(NPBF16)
        bqkv = np.concatenate(
            [np.asarray(bb, np.float32)[cols] for bb in (bq, bk, bv)]
        ).astype(np.float32)
        in_maps.append(
            {"xt": xts[s], "wqkv": wqkv, "bqkv": bqkv}
        )
    return in_maps


def make_wo_pad(Wo, g):
    wo_g = np.asarray(Wo, np.float32)[g * C : (g + 1) * C, :]  # [320, 1280]
    wo_pad = np.zeros((3 * P, D), np.float32)
    wo_pad[:C] = wo_g
    return wo_pad.astype(NPBF16)


_PROGRAM = None


def kernel(hidden_states, Wq, Wk, Wv, Wo, bq, bk, bv, bo):
    global _PROGRAM
    if _PROGRAM is None:
        _PROGRAM = build_program()
    nc = _PROGRAM

    in_maps = make_in_maps(hidden_states, Wq, Wk, Wv, bq, bk, bv)
    for c in range(N_CORES):
        in_maps[c]["wo"] = make_wo_pad(Wo, c % GROUPS)

    res = bass_utils.run_bass_kernel_spmd(nc, in_maps, core_ids=list(range(N_CORES)))
    hs = np.asarray(hidden_states, np.float32)
    bo = np.asarray(bo, np.float32)
    out = np.empty((BF, S, D), np.float32)
    for s in range(B):
        acc = np.zeros((N_SET, D), np.float32)
        for g in range(GROUPS):
            acc += np.asarray(res.results[s * GROUPS + g]["out"], np.float32)
        out[s * NUM_FRAMES : (s + 1) * NUM_FRAMES] = (
            acc.reshape(NUM_FRAMES, S, D)
            + bo[None, None, :]
            + hs[s * NUM_FRAMES : (s + 1) * NUM_FRAMES]
        )
    return out



# revision 9
# speedup vs baseline: 1.5694x; 1.5694x over previous
"""Trainium2 Bass kernel for nn_ConsistentSelfAttentionProcessor.

Reference computation (per frame-set of NUM_FRAMES=4 frames):
    q,k,v = hs@Wq+bq, hs@Wk+bk, hs@Wv+bv          # [BF,S,D]
    per head: K_comb = [K(frame0_of_set); K(own)]  # 2S keys
    out = softmax(q@K_comb^T/sqrt(hd)) @ V_comb @ Wo + bo + hs

Sharding: 8 cores = 2 frame-sets x 4 head-groups of 5 heads.
Each core computes a partial output  attn(set, heads_g) @ Wo[rows_g];
the host sums the 4 per-set partials in fp32 and adds bo + residual.

Structure (v3):
  - Q^T/K^T produced DIRECTLY via W-stationary matmuls (lhsT = Wq/Wk column
    chunks, rhs = X^T), no PE transposes.  qkt chunk layout: ch 0-2 = q head
    pairs (0,1),(2,3),(4,4); ch 3-5 = k same.  QK^T runs in bf16.
  - Projections, probs@V and O-proj run as fp8e4 DoubleRow matmuls (two
    128-partition contraction tiles summed per pass, ~1.4x PE throughput).
    attn values are scaled x32 into fp8 normal range (folded into the
    denominator broadcast), un-scaled in the O-proj output copy.
  - Softmax denominator via ones-column appended to V (ut row 64).
    Normalize tail: ut copied PSUM->SBUF, raw denominator broadcast across
    64 partitions by a K=1 PE matmul, reciprocal_approx_fast on [64,512]
    (InstReciprocal is ~6.5ns/row - 12x slower than a copy), multiply.
    Tail of head h is emitted after head h+1's first kc pair so the PE
    never stalls on it.
  - Projection chunks for frame f+1 and O-proj token chunks are paced as
    filler inside the attention loops so PE stays dense while ACT does exp
    (dense PE keeps the HAM clock throttle at 8/8).
Frame 0 of each set attends to [K0;K0] == softmax over K0 alone: 1024 keys.
Softmax uses no max subtraction: scores*0.125 is bounded (~|3|).
"""

import sys
from collections import deque
from contextlib import ExitStack

import numpy as np

sys.path.insert(0, "/opt/trn_rl_repo")

import ml_dtypes  # noqa: E402

import concourse.mybir as mybir  # noqa: E402
import concourse.tile as tile  # noqa: E402
from concourse import bacc, bass_utils  # noqa: E402
from concourse.masks import make_identity  # noqa: E402

BF16 = mybir.dt.bfloat16
F32 = mybir.dt.float32
FP8 = mybir.dt.float8e4
NPBF16 = ml_dtypes.bfloat16
NPFP8 = ml_dtypes.float8_e4m3
DR = mybir.MatmulPerfMode.DoubleRow

NUM_FRAMES = 4
HEADS = 20
BF, S, D = 8, 1024, 1280
HD = 64  # head dim
B = BF // NUM_FRAMES  # 2 frame sets
N_CORES = 8
GROUPS = 4  # head groups per set
HG = HEADS // GROUPS  # 5 heads per group
C = HG * HD  # 320 columns per group
N_SET = NUM_FRAMES * S  # 4096 rows per set
SCALE = 1.0 / np.sqrt(HD)  # 0.125
OSCALE = 32.0  # attn scaled into fp8 normal range; un-scaled in O-proj copy

P = 128
KC = D // P  # 10 contraction chunks for projections
NG = N_SET // 512  # 8 groups of 512 tokens per set
VPAD = 80  # per-head V stride in vsb (65 used; 5*80 bytes is 16-aligned)


def build_kernel_body(ctx: ExitStack, tc: tile.TileContext, xt, wqk, wv, wo, bqk, bv, out):
    """Emit the per-core program.

    xt:   [D, N_SET]      fp8   (X^T for this set)
    wqk:  [D, 768]        fp8   (6 chunks of 128: q pairs (0,1),(2,3),(4,4);
                                 k pairs same)
    wv:   [D, C]          fp8
    wo:   [4*P, D]        fp8   (rows: head pairs (0,1),(2,3),(4,zero),zero)
    bqk:  [P, 6]          f32   (per-partition bias for qkt chunks)
    bv:   [C]             f32
    out:  [N_SET, D]      bf16  (partial output, unsummed, no bo/residual)
    """
    nc = tc.nc

    const = ctx.enter_context(tc.tile_pool(name="const", bufs=1))
    persist = ctx.enter_context(tc.tile_pool(name="persist", bufs=1))
    work = ctx.enter_context(tc.tile_pool(name="work", bufs=2))
    psum = ctx.enter_context(tc.tile_pool(name="psum", bufs=1, space="PSUM"))

    # ---- constants ----------------------------------------------------------
    ident = const.tile([P, P], BF16, tag="ident")
    make_identity(nc, ident)
    # scaled-ones row for the K=1 denominator-broadcast matmul: broadcasts
    # den/OSCALE so the reciprocal yields OSCALE/den
    bones = const.tile([P, HD], BF16, tag="bones")
    nc.gpsimd.memset(bones, 1.0 / OSCALE)
    ones1 = const.tile([1, P], F32, tag="ones1")
    nc.gpsimd.memset(ones1, 1.0)

    xcols = [None] * NG

    def dma_xcol(g):
        xcols[g] = work.tile([P, KC, 512], FP8, tag="xcol", bufs=2, name=f"xc{g}")
        nc.sync.dma_start(
            xcols[g], xt[:, g * 512 : (g + 1) * 512].rearrange("(c p) n -> p c n", p=P)
        )

    dma_xcol(0)
    wqk_sb = const.tile([P, KC, 768], FP8, tag="wqk")
    nc.sync.dma_start(wqk_sb, wqk.rearrange("(c p) n -> p c n", p=P))
    wv_sb = const.tile([P, KC, C], FP8, tag="wv")
    nc.sync.dma_start(wv_sb, wv.rearrange("(c p) n -> p c n", p=P))
    bqk_sb = const.tile([P, 6], F32, tag="bqk")
    nc.sync.dma_start(bqk_sb, bqk)
    bv_sb = const.tile([1, C], F32, tag="bv")
    nc.sync.dma_start(bv_sb, bv[None, :])
    wo_sb = const.tile([P, 4, D], FP8, tag="wo")

    # broadcast V bias across partitions once: bv_bc[p, j] = bv[j]
    bv_bc = const.tile([P, C], F32, tag="bv_bc")
    bps = psum.tile([P, C], F32, tag="M", bufs=2)
    nc.tensor.matmul(bps, ones1, bv_sb)
    nc.vector.tensor_copy(bv_bc, bps)

    # ---- persistent intermediates ------------------------------------------
    # Q^T/K^T: chunk layout ch 0-2 q pairs, 3-5 k pairs; head h at base
    # partition (h%2)*64, chunk h//2 (+3 for k). head 4 duplicated.
    qkt = persist.tile([P, 6, N_SET], BF16, tag="qkt")
    # V rows with a ones column per head: [tokens, head, 65] (pad to 80)
    vsb = persist.tile([P, N_SET // P, HG, VPAD], FP8, tag="vsb")
    for h in range(HG):
        nc.gpsimd.memset(vsb[:, :, h, HD], 1.0)
    # attn^T (x OSCALE) for O-proj, one tensor per frame: chunk c holds heads
    # (2c,2c+1); chunk 2 top half and chunk 3 are zero (DoubleRow pairing).
    atn_f = [
        persist.tile([P, 4, S], FP8, tag=f"atn{f}", name=f"atn{f}")
        for f in range(NUM_FRAMES)
    ]
    for f in range(NUM_FRAMES):
        nc.gpsimd.memset(atn_f[f][HD:P, 2, :], 0.0)
        nc.gpsimd.memset(atn_f[f][:, 3, :], 0.0)

    # ---- projection / O-proj work units -------------------------------------
    def proj_qk(g, ch):
        """qkt[:, ch, g*512:(g+1)*512] = (W_ch^T X + b) for one 512-tok group."""
        if ch == 0 and g + 1 < NG:
            dma_xcol(g + 1)
        ps = psum.tile([P, 512], F32, tag="M", bufs=2)
        for kp in range(KC // 2):
            nc.tensor.matmul(
                ps,
                wqk_sb[:, 2 * kp : 2 * kp + 2, ch * P : (ch + 1) * P],
                xcols[g][:, 2 * kp : 2 * kp + 2, :],
                start=(kp == 0),
                stop=(kp == KC // 2 - 1),
                perf_mode=DR,
            )
        nc.vector.tensor_scalar(
            qkt[:, ch, g * 512 : (g + 1) * 512],
            ps,
            bqk_sb[:, ch : ch + 1],
            None,
            mybir.AluOpType.add,
        )

    def proj_v(g, sub):
        """vsb[:, t, :, 0:64] for 128-token chunk t = g*4+sub."""
        t = g * 4 + sub
        ps = psum.tile([P, C], F32, tag="M", bufs=2)
        for kp in range(KC // 2):
            nc.tensor.matmul(
                ps,
                xcols[g][:, 2 * kp : 2 * kp + 2, sub * P : (sub + 1) * P],
                wv_sb[:, 2 * kp : 2 * kp + 2, :],
                start=(kp == 0),
                stop=(kp == KC // 2 - 1),
                perf_mode=DR,
            )
        nc.vector.tensor_tensor(
            vsb[:, t, :, 0:HD],
            ps.rearrange("p (h d) -> p h d", d=HD),
            bv_bc.rearrange("p (h d) -> p h d", d=HD),
            mybir.AluOpType.add,
        )

    def oproj(f, tl):
        """out rows for token chunk tl of frame f (bf16 partial, /OSCALE)."""
        t = f * (S // P) + tl
        ou = work.tile([P, D], BF16, tag="ou", bufs=2)
        for n3, nw in ((0, 512), (1, 512), (2, 256)):
            po = psum.tile([P, 512], F32, tag="M", bufs=2)
            for cp in range(2):
                nc.tensor.matmul(
                    po[:, 0:nw],
                    atn_f[f][:, 2 * cp : 2 * cp + 2, tl * P : (tl + 1) * P],
                    wo_sb[:, 2 * cp : 2 * cp + 2, n3 * 512 : n3 * 512 + nw],
                    start=(cp == 0),
                    stop=(cp == 1),
                    perf_mode=DR,
                )
            nc.vector.tensor_scalar(
                ou[:, n3 * 512 : n3 * 512 + nw], po[:, 0:nw],
                1.0 / OSCALE, None, mybir.AluOpType.mult,
            )
        nc.sync.dma_start(out[t * P : (t + 1) * P, :], ou)

    # fill queue: (group_deadline_label, fn); proj for frame f must be
    # emitted before frame f's attention reads it
    fill = deque()

    def pop_fill(n=1):
        for _ in range(n):
            if fill:
                fill.popleft()[1]()

    def drain_fill_through_group(gmax):
        while fill and fill[0][0] is not None and fill[0][0] <= gmax:
            fill.popleft()[1]()

    # ---- softmax tail --------------------------------------------------------
    def tail(f, h, ut_sb):
        """atn_f[f] head h = OSCALE * ut_sb[0:64] / ut_sb[64] (denom row)."""
        for q in range(2):
            qs = slice(q * 512, (q + 1) * 512)
            bcp = psum.tile([HD, 512], F32, tag="M", bufs=2)
            nc.tensor.matmul(bcp, bones[HD : HD + 1, :], ut_sb[HD : HD + 1, qs])
            rc = work.tile([HD, 512], F32, tag="rc", bufs=2)
            nc.vector.reciprocal_approx_fast(out=rc, in_=bcp)
            if h % 2 == 0:
                nc.vector.tensor_tensor(
                    atn_f[f][0:HD, h // 2, qs], ut_sb[0:HD, qs], rc,
                    mybir.AluOpType.mult,
                )
            else:
                # result must land at partitions 64-127: mult to a base-0
                # tmp, then PE-copy shifts partitions
                tm = work.tile([HD, 512], BF16, tag="tm", bufs=2)
                nc.vector.tensor_tensor(tm, ut_sb[0:HD, qs], rc, mybir.AluOpType.mult)
                pc = psum.tile([P, 512], F32, tag="M", bufs=2)
                nc.tensor.matmul(pc[HD:P, :], ident[0:HD, 0:HD], tm)
                nc.vector.tensor_copy(atn_f[f][HD:P, h // 2, qs], pc[HD:P, :])

    # ---- emission ------------------------------------------------------------
    # frame 0 projections upfront (attention f0 needs them)
    for g in range(2):
        for ch in range(6):
            proj_qk(g, ch)
        for sub in range(4):
            proj_v(g, sub)
    nc.sync.dma_start(wo_sb, wo.rearrange("(c p) n -> p c n", p=P))
    # projections for frame f run as filler during frame f-1
    for g in range(2, NG):
        for ch in range(6):
            fill.append((g, lambda g=g, ch=ch: proj_qk(g, ch)))
        for sub in range(4):
            fill.append((g, lambda g=g, sub=sub: proj_v(g, sub)))

    pending = None  # deferred tail
    for f in range(NUM_FRAMES):
        qoff = f * S
        nkc = 8 if f == 0 else 16  # frame 0: ref==own, dedup
        if f >= 1:
            drain_fill_through_group(2 * f + 1)
        if f < NUM_FRAMES - 1:
            avail = sum(1 for lb, _ in fill if lb is None or lb <= 2 * f + 3)
        else:
            avail = len(fill)
        nkcp_f = HG * nkc // 2
        popped = 0
        for h in range(HG):
            b = (h % 2) * HD
            qch = h // 2
            kch = 3 + h // 2
            ut = psum.tile([HD + 1, S], F32, tag="ut", bufs=1)
            for kcp in range(nkc // 2):
                ex2 = work.tile([P, 2, S], FP8, tag="ex2", bufs=3)
                for par in range(2):
                    kc = 2 * kcp + par
                    ktok = kc * P if kc < 8 else qoff + (kc - 8) * P
                    sc = psum.tile([P, S], F32, tag="sc", bufs=2)
                    for q in range(2):
                        nc.tensor.matmul(
                            sc[:, q * 512 : (q + 1) * 512],
                            qkt[b : b + HD, kch, ktok : ktok + P],
                            qkt[b : b + HD, qch, qoff + q * 512 : qoff + (q + 1) * 512],
                        )
                    nc.scalar.activation(
                        ex2[:, par, :], sc, mybir.ActivationFunctionType.Exp,
                        scale=SCALE,
                    )
                # filler between QK and PV so PE overlaps the exp latency
                ikcp = h * (nkc // 2) + kcp
                if kcp == 1 and pending is not None:
                    tail(*pending)
                    pending = None
                want = (avail * (ikcp + 1)) // nkcp_f
                pop_fill(want - popped)
                popped = want
                # probs @ [V | ones]: fp8 DoubleRow over the kc pair
                t0 = 2 * kcp if kcp < 4 else f * 8 + 2 * (kcp - 4)
                for q in range(2):
                    nc.tensor.matmul(
                        ut[:, q * 512 : (q + 1) * 512],
                        vsb[:, t0 : t0 + 2, h, 0 : HD + 1],
                        ex2[:, :, q * 512 : (q + 1) * 512],
                        start=(kcp == 0),
                        stop=(kcp == nkc // 2 - 1),
                        perf_mode=DR,
                    )
            # stage ut to SBUF so PSUM frees and the tail can be deferred
            # (bf16: numerator ~0.4% rel err, diluted ~50x in the final out)
            ut_sb = work.tile([HD + 1, S], BF16, tag="utsb", bufs=2)
            nc.vector.tensor_copy(ut_sb, ut)
            pending = (f, h, ut_sb)
        # frame done: flush the last head's tail now so O-proj can run
        tail(*pending)
        pending = None
        for tl in range(S // P):
            fill.append((None, lambda f=f, tl=tl: oproj(f, tl)))
    pop_fill(len(fill))


def build_program():
    from concourse.bass_interp import get_hw_module

    nc = bacc.Bacc(
        "TRN2",
        target_bir_lowering=False,
        debug=False,
        enable_asserts=False,
        num_devices=N_CORES,
    )
    xt = nc.dram_tensor("xt", [D, N_SET], FP8, kind="ExternalInput").ap()
    wqk = nc.dram_tensor("wqk", [D, 768], FP8, kind="ExternalInput").ap()
    wv = nc.dram_tensor("wv", [D, C], FP8, kind="ExternalInput").ap()
    wo = nc.dram_tensor("wo", [4 * P, D], FP8, kind="ExternalInput").ap()
    bqk = nc.dram_tensor("bqk", [P, 6], F32, kind="ExternalInput").ap()
    bv = nc.dram_tensor("bv", [C], F32, kind="ExternalInput").ap()
    out = nc.dram_tensor("out", [N_SET, D], BF16, kind="ExternalOutput").ap()
    with tile.TileContext(nc) as tc:
        with ExitStack() as ctx:
            build_kernel_body(ctx, tc, xt, wqk, wv, wo, bqk, bv, out)
    nc.finalize()
    nc.m = get_hw_module(nc.m)
    return nc


def make_in_maps(hidden_states, Wq, Wk, Wv, bq, bk, bv):
    """Per-core inputs. Core c = set (c//4), head group (c%4)."""
    hs = np.asarray(hidden_states, np.float32).reshape(BF, S, D)
    Wq = np.asarray(Wq, np.float32)
    Wk = np.asarray(Wk, np.float32)
    bq = np.asarray(bq, np.float32)
    bk = np.asarray(bk, np.float32)
    xts = []
    for s in range(B):
        x = hs[s * NUM_FRAMES : (s + 1) * NUM_FRAMES].reshape(N_SET, D)
        xts.append(np.ascontiguousarray(x.T).astype(NPFP8))
    in_maps = []
    for c in range(N_CORES):
        s, g = c // GROUPS, c % GROUPS
        cols = slice(g * C, (g + 1) * C)
        wq_g, wk_g = Wq[:, cols], Wk[:, cols]
        bq_g, bk_g = bq[cols], bk[cols]
        # chunks: q pairs (0,1),(2,3),(4,4); k same
        chunks, bias_cols = [], []
        for W, bb in ((wq_g, bq_g), (wk_g, bk_g)):
            chunks += [W[:, 0:128], W[:, 128:256],
                       np.concatenate([W[:, 256:320], W[:, 256:320]], axis=1)]
            bias_cols += [bb[0:128], bb[128:256],
                          np.concatenate([bb[256:320], bb[256:320]])]
        wqk = np.concatenate(chunks, axis=1).astype(NPFP8)  # [D, 768]
        bqk = np.stack(bias_cols, axis=1).astype(np.float32)  # [128, 6]
        wv_c = np.asarray(Wv, np.float32)[:, cols].astype(NPFP8)
        bv_c = np.asarray(bv, np.float32)[cols].astype(np.float32)
        in_maps.append(
            {"xt": xts[s], "wqk": wqk, "bqk": bqk, "wv": wv_c, "bv": bv_c}
        )
    return in_maps


def make_wo_pad(Wo, g):
    wo_g = np.asarray(Wo, np.float32)[g * C : (g + 1) * C, :]  # [320, 1280]
    wo_pad = np.zeros((4 * P, D), np.float32)
    wo_pad[:C] = wo_g
    return wo_pad.astype(NPFP8)


_PROGRAM = None


def kernel(hidden_states, Wq, Wk, Wv, Wo, bq, bk, bv, bo):
    global _PROGRAM
    if _PROGRAM is None:
        _PROGRAM = build_program()
    nc = _PROGRAM

    in_maps = make_in_maps(hidden_states, Wq, Wk, Wv, bq, bk, bv)
    for c in range(N_CORES):
        in_maps[c]["wo"] = make_wo_pad(Wo, c % GROUPS)

    res = bass_utils.run_bass_kernel_spmd(nc, in_maps, core_ids=list(range(N_CORES)))
    hs = np.asarray(hidden_states, np.float32)
    bo = np.asarray(bo, np.float32)
    out = np.empty((BF, S, D), np.float32)
    for s in range(B):
        acc = np.zeros((N_SET, D), np.float32)
        for g in range(GROUPS):
            acc += np.asarray(res.results[s * GROUPS + g]["out"], np.float32)
        out[s * NUM_FRAMES : (s + 1) * NUM_FRAMES] = (
            acc.reshape(NUM_FRAMES, S, D)
            + bo[None, None, :]
            + hs[s * NUM_FRAMES : (s + 1) * NUM_FRAMES]
        )
    return out


# revision 16
# speedup vs baseline: 1.8278x; 1.1647x over previous
"""Trainium2 Bass kernel for nn_ConsistentSelfAttentionProcessor.

Reference computation (per frame-set of NUM_FRAMES=4 frames):
    q,k,v = hs@Wq+bq, hs@Wk+bk, hs@Wv+bv          # [BF,S,D]
    per head: K_comb = [K(frame0_of_set); K(own)]  # 2S keys
    out = softmax(q@K_comb^T/sqrt(hd)) @ V_comb @ Wo + bo + hs

Sharding: 8 cores = 2 frame-sets x 4 head-groups of 5 heads.
Each core computes a partial output  attn(set, heads_g) @ Wo[rows_g];
the host sums the 4 per-set partials in fp32 and adds bo + residual.

Structure (v3):
  - Q^T/K^T produced DIRECTLY via W-stationary matmuls (lhsT = Wq/Wk column
    chunks, rhs = X^T), no PE transposes.  qkt chunk layout: ch 0-2 = q head
    pairs (0,1),(2,3),(4,4); ch 3-5 = k same.  QK^T runs in bf16.
  - Projections, probs@V and O-proj run as fp8e4 DoubleRow matmuls (two
    128-partition contraction tiles summed per pass, ~1.4x PE throughput).
    attn values are scaled x32 into fp8 normal range (folded into the
    denominator broadcast), un-scaled in the O-proj output copy.
  - Softmax denominator via ones-column appended to V (ut row 64).
    Normalize tail: ut copied PSUM->SBUF, raw denominator broadcast across
    64 partitions by a K=1 PE matmul, reciprocal_approx_fast on [64,512]
    (InstReciprocal is ~6.5ns/row - 12x slower than a copy), multiply.
    Tail of head h is emitted after head h+1's first kc pair so the PE
    never stalls on it.
  - Projection chunks for frame f+1 and O-proj token chunks are paced as
    filler inside the attention loops so PE stays dense while ACT does exp
    (dense PE keeps the HAM clock throttle at 8/8).
Frame 0 of each set attends to [K0;K0] == softmax over K0 alone: 1024 keys.
Softmax uses no max subtraction: scores*0.125 is bounded (~|3|).
"""

import sys
from collections import deque
from contextlib import ExitStack

import numpy as np

sys.path.insert(0, "/opt/trn_rl_repo")

import ml_dtypes  # noqa: E402

import concourse.mybir as mybir  # noqa: E402
import concourse.tile as tile  # noqa: E402
from concourse import bacc, bass_utils  # noqa: E402
from concourse.masks import make_identity  # noqa: E402

BF16 = mybir.dt.bfloat16
F32 = mybir.dt.float32
FP8 = mybir.dt.float8e4
NPBF16 = ml_dtypes.bfloat16
NPFP8 = ml_dtypes.float8_e4m3
DR = mybir.MatmulPerfMode.DoubleRow

NUM_FRAMES = 4
HEADS = 20
BF, S, D = 8, 1024, 1280
HD = 64  # head dim
B = BF // NUM_FRAMES  # 2 frame sets
N_CORES = 8
GROUPS = 4  # head groups per set
HG = HEADS // GROUPS  # 5 heads per group
C = HG * HD  # 320 columns per group
N_SET = NUM_FRAMES * S  # 4096 rows per set
SCALE = 1.0 / np.sqrt(HD)  # 0.125
OSCALE = 32.0  # attn scaled into fp8 normal range; un-scaled in O-proj copy

P = 128
KC = D // P  # 10 contraction chunks for projections
NG = N_SET // 512  # 8 groups of 512 tokens per set
VPAD = 80  # per-head V stride in vsb (65 used; 5*80 bytes is 16-aligned)

# exp offload: these kc pairs (frames>=1, own-frame keys) compute exp on DVE
# via the bf16 bit trick  bf16_bits(exp(s)) ~ round(s*0.125*128/ln2 + 127*128)
# (2^frac approximated linearly by the mantissa: <=6% rel err, fine here).
DVE_KCP = (4, 6)
EXP_A = float(SCALE * P / np.log(2.0))
EXP_B = 127.0 * P


def build_kernel_body(ctx: ExitStack, tc: tile.TileContext, xt, wqk, wv, wo, bqk, bv, out):
    """Emit the per-core program.

    xt:   [D, N_SET]      fp8   (X^T for this set)
    wqk:  [D, 768]        fp8   (6 chunks of 128: q pairs (0,1),(2,3),(4,4);
                                 k pairs same)
    wv:   [D, C]          fp8
    wo:   [4*P, D]        fp8   (rows: head pairs (0,1),(2,3),(4,zero),zero)
    bqk:  [P, 6]          f32   (per-partition bias for qkt chunks)
    bv:   [C]             f32
    out:  [N_SET, D]      bf16  (partial output, unsummed, no bo/residual)
    """
    nc = tc.nc

    const = ctx.enter_context(tc.tile_pool(name="const", bufs=1))
    persist = ctx.enter_context(tc.tile_pool(name="persist", bufs=1))
    work = ctx.enter_context(tc.tile_pool(name="work", bufs=2))
    psum = ctx.enter_context(tc.tile_pool(name="psum", bufs=1, space="PSUM"))

    # ---- constants ----------------------------------------------------------
    ident = const.tile([P, P], BF16, tag="ident")
    make_identity(nc, ident)
    # scaled-ones row for the K=1 denominator-broadcast matmul: broadcasts
    # den/OSCALE so the reciprocal yields OSCALE/den
    bones = const.tile([P, HD], BF16, tag="bones")
    nc.gpsimd.memset(bones, 1.0 / OSCALE)
    ones1 = const.tile([1, P], F32, tag="ones1")
    nc.gpsimd.memset(ones1, 1.0)

    xcols = [None] * NG

    def dma_xcol(g):
        xcols[g] = work.tile([P, KC, 512], FP8, tag="xcol", bufs=2, name=f"xc{g}")
        nc.sync.dma_start(
            xcols[g], xt[:, g * 512 : (g + 1) * 512].rearrange("(c p) n -> p c n", p=P)
        )

    dma_xcol(0)
    wqk_sb = const.tile([P, KC, 768], FP8, tag="wqk")
    nc.sync.dma_start(wqk_sb, wqk.rearrange("(c p) n -> p c n", p=P))
    wv_sb = const.tile([P, KC, C], FP8, tag="wv")
    nc.sync.dma_start(wv_sb, wv.rearrange("(c p) n -> p c n", p=P))
    bqk_sb = const.tile([P, 6], F32, tag="bqk")
    nc.sync.dma_start(bqk_sb, bqk)
    bv_sb = const.tile([1, C], F32, tag="bv")
    nc.sync.dma_start(bv_sb, bv[None, :])
    wo_sb = const.tile([P, 4, D], FP8, tag="wo")

    # broadcast V bias across partitions once: bv_bc[p, j] = bv[j]
    bv_bc = const.tile([P, C], F32, tag="bv_bc")
    bps = psum.tile([P, C], F32, tag="M", bufs=2)
    nc.tensor.matmul(bps, ones1, bv_sb)
    nc.vector.tensor_copy(bv_bc, bps)

    # ---- persistent intermediates ------------------------------------------
    # Q^T/K^T: chunk layout ch 0-2 q pairs, 3-5 k pairs; head h at base
    # partition (h%2)*64, chunk h//2 (+3 for k). head 4 duplicated.
    qkt = persist.tile([P, 6, N_SET], BF16, tag="qkt")
    # V rows with a ones column per head: [tokens, head, 65] (pad to 80)
    vsb = persist.tile([P, N_SET // P, HG, VPAD], FP8, tag="vsb")
    for h in range(HG):
        nc.gpsimd.memset(vsb[:, :, h, HD], 1.0)
    # bf16 V copies for the DVE-routed kc chunks (their probs are bf16-bitcast)
    v16slot = {}
    for f in range(1, NUM_FRAMES):
        for kcp in DVE_KCP:
            for j in range(2):
                v16slot[f * 8 + 2 * (kcp - 4) + j] = len(v16slot)
    vsb16 = persist.tile([P, len(v16slot), HG, 66], BF16, tag="vsb16")
    for h in range(HG):
        nc.gpsimd.memset(vsb16[:, :, h, HD], 1.0)
    # attn^T (x OSCALE) for O-proj, one tensor per frame: chunk c holds heads
    # (2c,2c+1); chunk 2 top half and chunk 3 are zero (DoubleRow pairing).
    atn_f = [
        persist.tile([P, 4, S], FP8, tag=f"atn{f}", name=f"atn{f}")
        for f in range(NUM_FRAMES)
    ]
    for f in range(NUM_FRAMES):
        nc.gpsimd.memset(atn_f[f][HD:P, 2, :], 0.0)
        nc.gpsimd.memset(atn_f[f][:, 3, :], 0.0)

    # ---- projection / O-proj work units -------------------------------------
    def proj_qk(g, ch):
        """qkt[:, ch, g*512:(g+1)*512] = (W_ch^T X + b) for one 512-tok group."""
        if ch == 0 and g + 1 < NG:
            dma_xcol(g + 1)
        ps = psum.tile([P, 512], F32, tag="M", bufs=2)
        for kp in range(KC // 2):
            nc.tensor.matmul(
                ps,
                wqk_sb[:, 2 * kp : 2 * kp + 2, ch * P : (ch + 1) * P],
                xcols[g][:, 2 * kp : 2 * kp + 2, :],
                start=(kp == 0),
                stop=(kp == KC // 2 - 1),
                perf_mode=DR,
            )
        nc.vector.tensor_scalar(
            qkt[:, ch, g * 512 : (g + 1) * 512],
            ps,
            bqk_sb[:, ch : ch + 1],
            None,
            mybir.AluOpType.add,
        )

    def proj_v(g, sub):
        """vsb[:, t, :, 0:64] for 128-token chunk t = g*4+sub."""
        t = g * 4 + sub
        ps = psum.tile([P, C], F32, tag="M", bufs=2)
        for kp in range(KC // 2):
            nc.tensor.matmul(
                ps,
                xcols[g][:, 2 * kp : 2 * kp + 2, sub * P : (sub + 1) * P],
                wv_sb[:, 2 * kp : 2 * kp + 2, :],
                start=(kp == 0),
                stop=(kp == KC // 2 - 1),
                perf_mode=DR,
            )
        dst = (
            vsb16[:, v16slot[t], :, 0:HD] if t in v16slot else vsb[:, t, :, 0:HD]
        )
        nc.vector.tensor_tensor(
            dst,
            ps.rearrange("p (h d) -> p h d", d=HD),
            bv_bc.rearrange("p (h d) -> p h d", d=HD),
            mybir.AluOpType.add,
        )

    def oproj(f, tl):
        """out rows for token chunk tl of frame f (bf16 partial, /OSCALE)."""
        t = f * (S // P) + tl
        ou = work.tile([P, D], BF16, tag="ou", bufs=2)
        for n3, nw in ((0, 512), (1, 512), (2, 256)):
            po = psum.tile([P, 512], F32, tag="M", bufs=2)
            for cp in range(2):
                nc.tensor.matmul(
                    po[:, 0:nw],
                    atn_f[f][:, 2 * cp : 2 * cp + 2, tl * P : (tl + 1) * P],
                    wo_sb[:, 2 * cp : 2 * cp + 2, n3 * 512 : n3 * 512 + nw],
                    start=(cp == 0),
                    stop=(cp == 1),
                    perf_mode=DR,
                )
            nc.vector.tensor_scalar(
                ou[:, n3 * 512 : n3 * 512 + nw], po[:, 0:nw],
                1.0 / OSCALE, None, mybir.AluOpType.mult,
            )
        nc.sync.dma_start(out[t * P : (t + 1) * P, :], ou)

    # fill queue: (group_deadline_label, fn); proj for frame f must be
    # emitted before frame f's attention reads it
    fill = deque()

    def pop_fill(n=1):
        for _ in range(n):
            if fill:
                fill.popleft()[1]()

    def drain_fill_through_group(gmax):
        while fill and fill[0][0] is not None and fill[0][0] <= gmax:
            fill.popleft()[1]()

    # ---- softmax tail --------------------------------------------------------
    def tail(f, h, ut_sb):
        """atn_f[f] head h = OSCALE * ut_sb[0:64] / ut_sb[64] (denom row)."""
        for q in range(2):
            qs = slice(q * 512, (q + 1) * 512)
            bcp = psum.tile([HD, 512], F32, tag="M", bufs=2)
            nc.tensor.matmul(bcp, bones[HD : HD + 1, :], ut_sb[HD : HD + 1, qs])
            rc = work.tile([HD, 512], F32, tag="rc", bufs=2)
            nc.vector.reciprocal_approx_fast(out=rc, in_=bcp)
            if h % 2 == 0:
                nc.vector.tensor_tensor(
                    atn_f[f][0:HD, h // 2, qs], ut_sb[0:HD, qs], rc,
                    mybir.AluOpType.mult,
                )
            else:
                # result must land at partitions 64-127: mult to a base-0
                # tmp, then PE-copy shifts partitions
                tm = work.tile([HD, 512], BF16, tag="tm", bufs=2)
                nc.vector.tensor_tensor(tm, ut_sb[0:HD, qs], rc, mybir.AluOpType.mult)
                pc = psum.tile([P, 512], F32, tag="M", bufs=2)
                nc.tensor.matmul(pc[HD:P, :], ident[0:HD, 0:HD], tm)
                nc.vector.tensor_copy(atn_f[f][HD:P, h // 2, qs], pc[HD:P, :])

    # ---- emission ------------------------------------------------------------
    # frame 0 projections upfront (attention f0 needs them)
    for g in range(2):
        for ch in range(6):
            proj_qk(g, ch)
        for sub in range(4):
            proj_v(g, sub)
    nc.sync.dma_start(wo_sb, wo.rearrange("(c p) n -> p c n", p=P))
    # projections for frame f run as filler during frame f-1
    for g in range(2, NG):
        for ch in range(6):
            fill.append((g, lambda g=g, ch=ch: proj_qk(g, ch)))
        for sub in range(4):
            fill.append((g, lambda g=g, sub=sub: proj_v(g, sub)))

    pending = None  # deferred tail
    for f in range(NUM_FRAMES):
        qoff = f * S
        nkc = 8 if f == 0 else 16  # frame 0: ref==own, dedup
        if f >= 1:
            drain_fill_through_group(2 * f + 1)
        if f < NUM_FRAMES - 1:
            avail = sum(1 for lb, _ in fill if lb is None or lb <= 2 * f + 3)
        else:
            avail = len(fill)
        nkcp_f = HG * nkc // 2
        popped = 0
        for h in range(HG):
            b = (h % 2) * HD
            qch = h // 2
            kch = 3 + h // 2
            ut = psum.tile([HD + 1, S], F32, tag="ut", bufs=1)
            pend_pv = []
            for kcp in range(nkc // 2):
                routed = f >= 1 and kcp in DVE_KCP
                t0 = 2 * kcp if kcp < 4 else f * 8 + 2 * (kcp - 4)
                if routed:
                    exi = work.tile([P, 2, S], mybir.dt.int16, tag="exi", bufs=2)
                else:
                    ex2 = work.tile([P, 2, S], FP8, tag="ex2", bufs=3)
                for par in range(2):
                    kc = 2 * kcp + par
                    ktok = kc * P if kc < 8 else qoff + (kc - 8) * P
                    sc = psum.tile([P, S], F32, tag="sc", bufs=2)
                    for q in range(2):
                        nc.tensor.matmul(
                            sc[:, q * 512 : (q + 1) * 512],
                            qkt[b : b + HD, kch, ktok : ktok + P],
                            qkt[b : b + HD, qch, qoff + q * 512 : qoff + (q + 1) * 512],
                        )
                    if routed:
                        nc.vector.tensor_scalar(
                            exi[:, par, :], sc, EXP_A, EXP_B,
                            mybir.AluOpType.mult, mybir.AluOpType.add,
                        )
                    else:
                        nc.scalar.activation(
                            ex2[:, par, :], sc, mybir.ActivationFunctionType.Exp,
                            scale=SCALE,
                        )
                # filler between QK and PV so PE overlaps the exp latency
                ikcp = h * (nkc // 2) + kcp
                if kcp == 1 and pending is not None:
                    tail(*pending)
                    pending = None
                want = (avail * (ikcp + 1)) // nkcp_f
                pop_fill(want - popped)
                popped = want
                # routed PV from the PREVIOUS kc pair (gives DVE time to
                # produce the bitcast probs without stalling the PE)
                for mm in pend_pv:
                    mm()
                pend_pv = []
                if routed:
                    def pv_routed(exi=exi, t0=t0, ut=ut, h=h):
                        for par in range(2):
                            for q in range(2):
                                nc.tensor.matmul(
                                    ut[:, q * 512 : (q + 1) * 512],
                                    vsb16[:, v16slot[t0 + par], h, 0 : HD + 1],
                                    exi[:, par, q * 512 : (q + 1) * 512].bitcast(BF16),
                                    start=False,
                                    stop=False,
                                )
                    pend_pv = [pv_routed]
                else:
                    # probs @ [V | ones]: fp8 DoubleRow over the kc pair
                    for q in range(2):
                        nc.tensor.matmul(
                            ut[:, q * 512 : (q + 1) * 512],
                            vsb[:, t0 : t0 + 2, h, 0 : HD + 1],
                            ex2[:, :, q * 512 : (q + 1) * 512],
                            start=(kcp == 0),
                            stop=(kcp == nkc // 2 - 1),
                            perf_mode=DR,
                        )
            # stage ut to SBUF so PSUM frees and the tail can be deferred
            # (bf16: numerator ~0.4% rel err, diluted ~50x in the final out)
            ut_sb = work.tile([HD + 1, S], BF16, tag="utsb", bufs=2)
            nc.vector.tensor_copy(ut_sb, ut)
            pending = (f, h, ut_sb)
        # frame done: flush the last head's tail now so O-proj can run
        tail(*pending)
        pending = None
        for tl in range(S // P):
            fill.append((None, lambda f=f, tl=tl: oproj(f, tl)))
    pop_fill(len(fill))


def build_program():
    from concourse.bass_interp import get_hw_module

    nc = bacc.Bacc(
        "TRN2",
        target_bir_lowering=False,
        debug=False,
        enable_asserts=False,
        num_devices=N_CORES,
    )
    xt = nc.dram_tensor("xt", [D, N_SET], FP8, kind="ExternalInput").ap()
    wqk = nc.dram_tensor("wqk", [D, 768], FP8, kind="ExternalInput").ap()
    wv = nc.dram_tensor("wv", [D, C], FP8, kind="ExternalInput").ap()
    wo = nc.dram_tensor("wo", [4 * P, D], FP8, kind="ExternalInput").ap()
    bqk = nc.dram_tensor("bqk", [P, 6], F32, kind="ExternalInput").ap()
    bv = nc.dram_tensor("bv", [C], F32, kind="ExternalInput").ap()
    out = nc.dram_tensor("out", [N_SET, D], BF16, kind="ExternalOutput").ap()
    with tile.TileContext(nc) as tc:
        with ExitStack() as ctx:
            build_kernel_body(ctx, tc, xt, wqk, wv, wo, bqk, bv, out)
    nc.finalize()
    nc.m = get_hw_module(nc.m)
    return nc


def make_in_maps(hidden_states, Wq, Wk, Wv, bq, bk, bv):
    """Per-core inputs. Core c = set (c//4), head group (c%4)."""
    hs = np.asarray(hidden_states, np.float32).reshape(BF, S, D)
    Wq = np.asarray(Wq, np.float32)
    Wk = np.asarray(Wk, np.float32)
    bq = np.asarray(bq, np.float32)
    bk = np.asarray(bk, np.float32)
    xts = []
    for s in range(B):
        x = hs[s * NUM_FRAMES : (s + 1) * NUM_FRAMES].reshape(N_SET, D)
        xts.append(np.ascontiguousarray(x.T).astype(NPFP8))
    in_maps = []
    for c in range(N_CORES):
        s, g = c // GROUPS, c % GROUPS
        cols = slice(g * C, (g + 1) * C)
        wq_g, wk_g = Wq[:, cols], Wk[:, cols]
        bq_g, bk_g = bq[cols], bk[cols]
        # chunks: q pairs (0,1),(2,3),(4,4); k same
        chunks, bias_cols = [], []
        for W, bb in ((wq_g, bq_g), (wk_g, bk_g)):
            chunks += [W[:, 0:128], W[:, 128:256],
                       np.concatenate([W[:, 256:320], W[:, 256:320]], axis=1)]
            bias_cols += [bb[0:128], bb[128:256],
                          np.concatenate([bb[256:320], bb[256:320]])]
        wqk = np.concatenate(chunks, axis=1).astype(NPFP8)  # [D, 768]
        bqk = np.stack(bias_cols, axis=1).astype(np.float32)  # [128, 6]
        wv_c = np.asarray(Wv, np.float32)[:, cols].astype(NPFP8)
        bv_c = np.asarray(bv, np.float32)[cols].astype(np.float32)
        in_maps.append(
            {"xt": xts[s], "wqk": wqk, "bqk": bqk, "wv": wv_c, "bv": bv_c}
        )
    return in_maps


def make_wo_pad(Wo, g):
    wo_g = np.asarray(Wo, np.float32)[g * C : (g + 1) * C, :]  # [320, 1280]
    wo_pad = np.zeros((4 * P, D), np.float32)
    wo_pad[:C] = wo_g
    return wo_pad.astype(NPFP8)


_PROGRAM = None


def kernel(hidden_states, Wq, Wk, Wv, Wo, bq, bk, bv, bo):
    global _PROGRAM
    if _PROGRAM is None:
        _PROGRAM = build_program()
    nc = _PROGRAM

    in_maps = make_in_maps(hidden_states, Wq, Wk, Wv, bq, bk, bv)
    for c in range(N_CORES):
        in_maps[c]["wo"] = make_wo_pad(Wo, c % GROUPS)

    res = bass_utils.run_bass_kernel_spmd(nc, in_maps, core_ids=list(range(N_CORES)))
    hs = np.asarray(hidden_states, np.float32)
    bo = np.asarray(bo, np.float32)
    out = np.empty((BF, S, D), np.float32)
    for s in range(B):
        acc = np.zeros((N_SET, D), np.float32)
        for g in range(GROUPS):
            acc += np.asarray(res.results[s * GROUPS + g]["out"], np.float32)
        out[s * NUM_FRAMES : (s + 1) * NUM_FRAMES] = (
            acc.reshape(NUM_FRAMES, S, D)
            + bo[None, None, :]
            + hs[s * NUM_FRAMES : (s + 1) * NUM_FRAMES]
        )
    return out
